# revision 22
# baseline (speedup 1.0000x reference)
"""GCN (4-layer message-passing + linear head) on 8 Trainium2 NeuronCores.

Strategy (per sharding hint): nodes are partitioned across the 8 cores
(degree-banded for load balance); edges are partitioned by destination node
so the segment-sum is core-local.  Per conv layer, each core computes the
linear transform for its own nodes, the scaled feature table is all-gathered
(the "halo exchange" degenerates to a full exchange for this random graph),
messages are fetched with descriptor-DMA row gathers (dma_gather) into a
destination-major padded slot layout, and the segment sum becomes a strided
free-axis reduction on the vector engine.  Weights are replicated.

Math note: with deg = indeg+1 (self loop) and dinv = deg^-1/2, the reference
layer is   y = relu(dinv*[sum_{e:dst=v} dinv[src] h[src]] + dinv^2 h[v] + b)
which we compute as  g = dinv*h ;  y = relu(dinv*(segsum(g[src]) + g[v]) + b).
"""

import os
import sys
import numpy as np

for _p in ("/opt/trn_rl_repo",):
    if os.path.isdir(_p) and _p not in sys.path:
        sys.path.insert(0, _p)

# ----------------------------------------------------------------------------
# Problem constants (hardcoded per contract)
# ----------------------------------------------------------------------------
N = 40000
E = 640000
F_IN = 128
H = 64
C_OUT = 32
M = 8                      # cores
LANES = 125                # real nodes per tile (lanes 125..127 are padding)
TILES = 40                 # 40 tiles * 125 lanes = 5000 nodes per core
NPC = LANES * TILES        # 5000 nodes per core
SH = NPC + 1               # shard rows in the all-gather input (+1 zero row)
TBL = SH * M               # 40008 table rows
VIEW_A = (0, 32768)
VIEW_B = (TBL - 32768, TBL)        # [7240, 40008)
ZROW_A = NPC                       # core 0 zero row (< 32768)
ZROW_B = 6 * SH + NPC              # core 6 zero row (35006, inside view B)
GROUP_BLOCK_BUDGET = 40            # max gather blocks per dma_gather pair
N_QUEUES = 4                       # SWDGE rings; round-robin gather calls
LAYER_DIMS = [(F_IN, H), (H, 2 * H), (2 * H, 2 * H), (2 * H, H)]


# ----------------------------------------------------------------------------
# CPU-side graph partitioning / sharding prep (pure index manipulation + the
# degree normalization constants)
# ----------------------------------------------------------------------------
def _prep(edge_index):
    src = np.asarray(edge_index[0], dtype=np.int64)
    dst = np.asarray(edge_index[1], dtype=np.int64)

    deg_in = np.bincount(dst, minlength=N)
    dinv = (1.0 / np.sqrt((deg_in + 1).astype(np.float32))).astype(np.float32)

    # global degree-sorted order; band b = ranks [1000b, 1000(b+1)) feeds tile
    # b on every core (125 nodes/core/band) so per-tile padding is uniform.
    order = np.argsort(-deg_in, kind="stable")
    ranks = np.empty(N, np.int64)
    ranks[order] = np.arange(N)
    node_core = ranks % M
    within = ranks // M                      # 0..4999 rank within core
    node_tile = within // LANES              # 0..39
    node_lane = within % LANES               # 0..124
    node_pos = node_lane * TILES + node_tile  # shard row (lane-major)
    node_row = node_core * SH + node_pos      # global table row

    r_src = node_row[src]
    forced_hi = r_src >= VIEW_A[1]
    forced_lo = r_src < VIEW_B[0]

    # per (core, tile, lane) counts of forced-lo / forced-hi / flex edges
    c_, t_, l_ = node_core[dst], node_tile[dst], node_lane[dst]
    lin = (c_ * TILES + t_) * 128 + l_
    nbins = M * TILES * 128
    cnt_a = np.bincount(lin[forced_lo], minlength=nbins).reshape(M, TILES, 128)
    cnt_b = np.bincount(lin[forced_hi], minlength=nbins).reshape(M, TILES, 128)
    cnt_t = np.bincount(lin, minlength=nbins).reshape(M, TILES, 128)
    cnt_f = cnt_t - cnt_a - cnt_b

    # choose per-band lo/hi slot counts (shared by all cores: SPMD program)
    NLO = np.zeros(TILES, np.int64)
    NHI = np.zeros(TILES, np.int64)
    L_t = np.zeros(TILES, np.int64)
    for t in range(TILES):
        at = cnt_a[:, t, :].ravel()
        ft = cnt_f[:, t, :].ravel()
        dt = cnt_t[:, t, :].ravel()
        best = None
        for L in range(int(dt.max()) + 1):
            lo = np.clip(L, at, at + ft)
            cost = (lo.max() + (dt - lo).max(), lo.max(), (dt - lo).max())
            if best is None or cost < best:
                best = cost
                L_t[t] = L
        NLO[t], NHI[t] = best[1], best[2]

    # greedy grouping of tiles into gather-call pairs under the block budget
    groups = []
    cur, acc = [], 0
    for t in range(TILES):
        b = int(NLO[t] + NHI[t])
        if cur and acc + b > GROUP_BLOCK_BUDGET:
            groups.append(cur)
            cur, acc = [], 0
        cur.append(t)
        acc += b
    groups.append(cur)

    # per-edge slot assignment.  Edge e (dst d): lane = node_lane[d], tile =
    # node_tile[d], goes lo if forced_lo, hi if forced_hi, else fills lo up to
    # clamp(L_t, a, a+f) then hi.
    lo_cap = np.clip(L_t[t_], cnt_a[c_, t_, l_], cnt_a[c_, t_, l_] + cnt_f[c_, t_, l_])
    # order edges per (c,t,l): forced_lo first, flex, forced_hi
    klass = np.where(forced_lo[np.arange(E)] if False else forced_lo, 0,
                     np.where(forced_hi, 2, 1))
    order_e = np.lexsort((klass, lin))
    lin_s = lin[order_e]
    # position of each edge within its (c,t,l) bucket, in sorted order
    uniq, start_idx, counts = np.unique(lin_s, return_index=True, return_counts=True)
    pos_in_bucket = np.arange(E) - np.repeat(start_idx, counts)
    is_lo_s = pos_in_bucket < lo_cap[order_e]
    slot_s = np.where(is_lo_s, pos_in_bucket, pos_in_bucket - lo_cap[order_e])

    # sanity: slots fit
    t_s = t_[order_e]
    assert (slot_s[is_lo_s] < NLO[t_s[is_lo_s]]).all()
    assert (slot_s[~is_lo_s] < NHI[t_s[~is_lo_s]]).all()

    # build per-core idx lists.  Call layout per group g: first the lo call
    # (blocks = sum NLO[t] over tiles of g, tile-major), then the hi call.
    blo_off = {}
    bhi_off = {}
    call_cols = []          # (group, which) -> (col0, nblocks)
    total_blocks = 0
    col0 = 0
    for gi, g in enumerate(groups):
        off = 0
        for t in g:
            blo_off[t] = total_blocks + off
            off += int(NLO[t])
        call_cols.append((col0, off))
        col0 += off * 8
        total_blocks += off
        off = 0
        for t in g:
            bhi_off[t] = total_blocks + off
            off += int(NHI[t])
        call_cols.append((col0, off))
        col0 += off * 8
        total_blocks += off

    W_COLS = total_blocks * 8

    idx_flat = np.empty((M, total_blocks * 128), np.int16)
    idx_flat[:, :] = 0
    # default fills: pad slots -> zero rows (view-relative)
    blk_is_lo = np.zeros(total_blocks, bool)
    for t in range(TILES):
        blk_is_lo[blo_off[t]:blo_off[t] + int(NLO[t])] = True
    pad_lo = np.int16(ZROW_A - VIEW_A[0])
    pad_hi = np.int16(ZROW_B - VIEW_B[0])
    for c in range(M):
        v = idx_flat[c].reshape(total_blocks, 128)
        v[blk_is_lo, :] = pad_lo
        v[~blk_is_lo, :] = pad_hi

    # scatter the real edges
    c_s, l_s = c_[order_e], l_[order_e]
    r_s = node_row[src[order_e]]
    base_blk = np.where(is_lo_s,
                        np.array([blo_off[t] for t in range(TILES)])[t_s],
                        np.array([bhi_off[t] for t in range(TILES)])[t_s])
    pos = (base_blk + slot_s) * 128 + l_s
    val = np.where(is_lo_s, r_s - VIEW_A[0], r_s - VIEW_B[0])
    assert val.min() >= 0 and val.max() <= 32767
    idx_flat[c_s, pos] = val.astype(np.int16)

    # wrapped [16, W] layout (idx i -> partition i%16, col i//16), x8 replicas
    idx_wrapped = np.empty((M, 128, W_COLS), np.int16)
    for c in range(M):
        w = idx_flat[c].reshape(W_COLS, 16).T
        idx_wrapped[c] = np.tile(w, (8, 1))

    # per-core dinv columns [128, TILES] (pad lanes get 0 -> zero g rows)
    dinv_col = np.zeros((M, 128, TILES), np.float32)
    tmp = dinv[order].reshape(NPC, M)  # rank-major: rows=within, cols=core? careful
    # node with rank r: core r%M, within r//M
    for c in range(M):
        nodes_c = order[c::M]          # within order 0..4999
        dc = dinv[nodes_c].reshape(TILES, LANES)  # within = tile*LANES + lane
        dinv_col[c, :LANES, :] = dc.T
    node_of = np.full((M, 128, TILES), -1, np.int64)
    for c in range(M):
        nodes_c = order[c::M]
        node_of[c, :LANES, :] = nodes_c.reshape(TILES, LANES).T

    return dict(
        dinv=dinv, node_core=node_core, node_pos=node_pos, node_of=node_of,
        NLO=NLO, NHI=NHI, groups=groups, blo_off=blo_off, bhi_off=bhi_off,
        call_cols=call_cols, total_blocks=total_blocks, W_COLS=W_COLS,
        idx_wrapped=idx_wrapped, dinv_col=dinv_col,
    )


# ----------------------------------------------------------------------------
# Bass/Tile program
# ----------------------------------------------------------------------------
def _build(prep, reps=1):
    import concourse.bass as bass
    import concourse.tile as tile
    from concourse import bacc, mybir

    NLO, NHI = prep["NLO"], prep["NHI"]
    groups, call_cols = prep["groups"], prep["call_cols"]
    blo_off, bhi_off = prep["blo_off"], prep["bhi_off"]
    W_COLS = prep["W_COLS"]
    f32 = mybir.dt.float32

    single = bool(os.environ.get("GNN_SINGLE"))
    nc = bacc.Bacc("TRN2", target_bir_lowering=False, debug=False,
                   num_devices=1 if single else M, num_swdge_queues=N_QUEUES)

    x_fm = nc.dram_tensor("x_fm", [128, TILES * 128], f32, kind="ExternalInput")
    idx_in = nc.dram_tensor("idx_in", [128, W_COLS], mybir.dt.int16, kind="ExternalInput")
    dinv_in = nc.dram_tensor("dinv_in", [128, TILES], f32, kind="ExternalInput")
    iden_in = nc.dram_tensor("iden_in", [128, 128], f32, kind="ExternalInput")
    W_in, B_in = [], []
    for k, (di, do) in enumerate(LAYER_DIMS):
        W_in.append(nc.dram_tensor(f"w{k+1}", [di, do], f32, kind="ExternalInput"))
        B_in.append(nc.dram_tensor(f"b{k+1}", [128, do], f32, kind="ExternalInput"))
    Wl_in = nc.dram_tensor("wl", [H, C_OUT], f32, kind="ExternalInput")
    bl_in = nc.dram_tensor("bl", [C_OUT, 1], f32, kind="ExternalInput")
    out_t = nc.dram_tensor("out_fm", [C_OUT, TILES * 128], f32, kind="ExternalOutput")

    bf16 = mybir.dt.bfloat16
    tdt = [bf16 if do == 128 else f32 for (_, do) in LAYER_DIMS]
    ag_in, table = [], []
    for k, (_, do) in enumerate(LAYER_DIMS):
        ag_in.append(nc.dram_tensor(f"ag_in{k+1}", [SH, do], tdt[k], kind="Internal"))
        table.append(nc.dram_tensor(f"table{k+1}", [TBL, do], tdt[k],
                                    kind="Internal", addr_space="Shared"))

    MAXD = 128

    with tile.TileContext(nc) as tc:
        import contextlib
        with contextlib.ExitStack() as ctx:
            const_p = ctx.enter_context(tc.tile_pool(name="const", bufs=1))
            state_p = ctx.enter_context(tc.tile_pool(name="state", bufs=1))
            work_p = ctx.enter_context(tc.tile_pool(name="work", bufs=1))
            gbuf_p = ctx.enter_context(tc.tile_pool(name="gbuf", bufs=6))
            pz_p = ctx.enter_context(tc.tile_pool(name="pz", bufs=3, space="PSUM"))
            small_p = ctx.enter_context(tc.tile_pool(name="small", bufs=2))
            pmm_p = ctx.enter_context(tc.tile_pool(name="pmm", bufs=2, space="PSUM"))
            ptr_p = ctx.enter_context(tc.tile_pool(name="ptr", bufs=3, space="PSUM"))

            # constants
            idx_t = const_p.tile([128, W_COLS], mybir.dt.int16)
            nc.sync.dma_start(idx_t[:, :], idx_in[:, :])
            dinv_t = const_p.tile([128, TILES], f32)
            nc.sync.dma_start(dinv_t[:, :], dinv_in[:, :])
            iden_t = const_p.tile([128, 128], f32)
            nc.sync.dma_start(iden_t[:, :], iden_in[:, :])
            iden_bf = const_p.tile([128, 128], bf16)
            nc.scalar.activation(iden_bf[:, :], iden_t[:, :],
                                 mybir.ActivationFunctionType.Copy)
            W_t, B_t = [], []
            for k, (di, do) in enumerate(LAYER_DIMS):
                w = const_p.tile([di, do], f32, tag=f"w{k}")
                nc.sync.dma_start(w[:, :], W_in[k][:, :])
                W_t.append(w)
                b = const_p.tile([128, do], f32, tag=f"b{k}")
                nc.sync.dma_start(b[:, :], B_in[k][:, :])
                B_t.append(b)
            wl_t = const_p.tile([H, C_OUT], f32, tag="wl")
            nc.sync.dma_start(wl_t[:, :], Wl_in[:, :])
            bl_t = const_p.tile([C_OUT, 1], f32, tag="bl")
            nc.sync.dma_start(bl_t[:, :], bl_in[:, :])

            for rep in range(reps):
                y_prev = None
                for k, (di, do) in enumerate(LAYER_DIMS):
                    # ---- X in feature-major form [di, 5120]
                    xfm = work_p.tile([128, TILES * 128], f32, tag="xfm")
                    if k == 0:
                        nc.sync.dma_start(xfm[:, :], x_fm[:, :])
                    elif not os.environ.get("GNN_SKIP_PE"):
                        for t in range(TILES):
                            ps = ptr_p.tile([128, 128], f32, tag="ptr")
                            nc.tensor.transpose(ps[0:di, :], y_prev[:, t, 0:di],
                                                iden_t[:, :])
                            nc.scalar.copy(xfm[0:di, t * 128:(t + 1) * 128],
                                           ps[0:di, :])
                    # ---- h = X @ W (feature-major out), evict to SBUF
                    hsb = work_p.tile([128, TILES * 128], f32, tag="hsb")
                    for n in range(0 if os.environ.get("GNN_SKIP_PE") else TILES * 128 // 512):
                        pm = pmm_p.tile([128, 512], f32, tag="pmm")
                        nc.tensor.matmul(pm[0:do, :], W_t[k][0:di, 0:do],
                                         xfm[0:di, n * 512:(n + 1) * 512])
                        nc.scalar.copy(hsb[0:do, n * 512:(n + 1) * 512],
                                       pm[0:do, :])
                    # ---- g = dinv * h, node-major [128, TILES, do]
                    g_nm = work_p.tile([128, TILES, do], f32, tag="g_nm")
                    if os.environ.get("GNN_SKIP_PE"):
                        nc.vector.memset(g_nm[:, :, :], 0.0)
                    for t in range(0 if os.environ.get("GNN_SKIP_PE") else TILES):
                        ps = ptr_p.tile([128, 128], f32, tag="ptr")
                        nc.tensor.transpose(ps[:, 0:do],
                                            hsb[0:do, t * 128:(t + 1) * 128],
                                            iden_t[0:do, 0:do])
                        nc.scalar.activation(
                            g_nm[:, t, 0:do], ps[:, 0:do],
                            mybir.ActivationFunctionType.Copy,
                            scale=dinv_t[:, t:t + 1])
                    # ---- publish shard (+ its zero row) and all-gather
                    if tdt[k] == bf16:
                        g_pub = work_p.tile([128, TILES, do], bf16, tag="g_pub")
                        nc.scalar.activation(g_pub[:, :, 0:do], g_nm[:, :, 0:do],
                                             mybir.ActivationFunctionType.Copy)
                    else:
                        g_pub = g_nm
                    nc.sync.dma_start(ag_in[k][0:NPC, :], g_pub[0:LANES, :, 0:do])
                    nc.sync.dma_start(ag_in[k][NPC:NPC + 1, :], g_pub[125:126, 0:1, 0:do])
                    if single:
                        nc.sync.dma_start(table[k][0:SH, :], ag_in[k][:, :])
                    elif not os.environ.get("GNN_SKIP_AG"):
                        nc.gpsimd.collective_compute(
                            "AllGather", mybir.AluOpType.bypass,
                            replica_groups=[list(range(M))],
                            ins=[ag_in[k][:, :]], outs=[table[k][:, :]],
                        )
                    # ---- gather + segment reduce
                    skip_gather = bool(os.environ.get("GNN_SKIP_GATHER"))
                    skip_reduce = skip_gather or bool(os.environ.get("GNN_SKIP_REDUCE"))
                    z = work_p.tile([128, TILES, do], f32, tag="z")
                    if skip_reduce:
                        nc.vector.memset(z[:, :, :], 0.0)
                    qctr = 0
                    view_a = table[k][VIEW_A[0]:VIEW_A[1], :]
                    view_b = table[k][VIEW_B[0]:VIEW_B[1], :]
                    gb_sz = max(sum(int(NLO[t]) + int(NHI[t]) for t in g)
                                for g in groups)
                    iden_k = iden_bf if tdt[k] == bf16 else iden_t
                    for gi, g in enumerate(groups):
                        nblo = sum(int(NLO[t]) for t in g)
                        nbhi = sum(int(NHI[t]) for t in g)
                        gb = gbuf_p.tile([128, gb_sz, do], tdt[k], tag="gb")
                        for which, (nb, view) in enumerate(
                                [(nblo, view_a), (nbhi, view_b)]):
                            col0, nbc = call_cols[2 * gi + which]
                            assert nbc == nb
                            if nb == 0 or skip_gather:
                                continue
                            off = 0 if which == 0 else nblo
                            nc.gpsimd.dma_gather(
                                gb[:, off:off + nb, 0:do], view,
                                idx_t[:, col0:col0 + nb * 8],
                                nb * 128, nb * 128, do,
                                single_packet=False, queue_num=qctr % N_QUEUES)
                            qctr += 1
                        base_lo = blo_off[g[0]]
                        base_hi = bhi_off[g[0]]
                        for t in g:
                            if skip_reduce:
                                continue
                            if t % 2 == 0:
                                # PE segment-sum: identity-matmul accumulate
                                # tile t's blocks into one PSUM tile.
                                blks = [blo_off[t] - base_lo + j
                                        for j in range(int(NLO[t]))]
                                blks += [nblo + (bhi_off[t] - base_hi) + j
                                         for j in range(int(NHI[t]))]
                                psz = pz_p.tile([128, do], f32, tag="psz")
                                for bi, b in enumerate(blks):
                                    nc.tensor.matmul(
                                        psz[:, 0:do], iden_k[:, :],
                                        gb[:, b, 0:do],
                                        start=(bi == 0),
                                        stop=(bi == len(blks) - 1))
                                nc.scalar.copy(z[:, t, 0:do], psz[:, 0:do])
                                continue
                            # DVE segment-sum for odd tiles (strided reduce)
                            o = blo_off[t] - base_lo
                            nc.vector.tensor_reduce(
                                z[:, t, 0:do],
                                gb[:, o:o + int(NLO[t]), 0:do]
                                .rearrange("p b d -> p d b"),
                                axis=mybir.AxisListType.X, op=mybir.AluOpType.add)
                            if int(NHI[t]) > 0:
                                o = nblo + (bhi_off[t] - base_hi)
                                zh = small_p.tile([128, do], f32, tag="zh")
                                nc.vector.tensor_reduce(
                                    zh[:, 0:do],
                                    gb[:, o:o + int(NHI[t]), 0:do]
                                    .rearrange("p b d -> p d b"),
                                    axis=mybir.AxisListType.X,
                                    op=mybir.AluOpType.add)
                                nc.vector.tensor_add(z[:, t, 0:do], z[:, t, 0:do],
                                                     zh[:, 0:do])
                    # ---- z += g_own ; y = relu(dinv*z + b)
                    nc.vector.tensor_add(z[:, :, 0:do], z[:, :, 0:do],
                                         g_nm[:, :, 0:do])
                    y = state_p.tile([128, TILES, do], f32, tag="y")
                    for t in range(TILES):
                        if os.environ.get("GNN_SKIP_TAIL"):
                            continue
                        nc.vector.tensor_scalar_mul(z[:, t, 0:do], z[:, t, 0:do],
                                                    dinv_t[:, t:t + 1])
                        nc.vector.tensor_add(z[:, t, 0:do], z[:, t, 0:do],
                                             B_t[k][:, 0:do])
                    nc.scalar.activation(y[:, :, 0:do], z[:, :, 0:do],
                                         mybir.ActivationFunctionType.Relu)
                    y_prev = y

                # ---- final linear head: out = y4 @ Wl + bl  (feature-major out)
                xfm = work_p.tile([128, TILES * 128], f32, tag="xfm")
                for t in range(TILES):
                    ps = ptr_p.tile([128, 128], f32, tag="ptr")
                    nc.tensor.transpose(ps[0:H, :], y_prev[:, t, 0:H], iden_t[:, :])
                    nc.scalar.copy(xfm[0:H, t * 128:(t + 1) * 128], ps[0:H, :])
                osb = work_p.tile([C_OUT, TILES * 128], f32, tag="hsb")
                for n in range(TILES * 128 // 512):
                    pm = pmm_p.tile([128, 512], f32, tag="pmm")
                    nc.tensor.matmul(pm[0:C_OUT, :], wl_t[0:H, 0:C_OUT],
                                     xfm[0:H, n * 512:(n + 1) * 512])
                    nc.vector.tensor_scalar_add(osb[:, n * 512:(n + 1) * 512],
                                                pm[0:C_OUT, :], bl_t[:, 0:1])
                nc.sync.dma_start(out_t[:, :], osb[:, :])

    nc.compile()
    return nc


def _in_maps(prep, inputs):
    x = np.asarray(inputs["x"], np.float32)
    maps = []
    for c in range(M):
        x_fm = np.zeros((128, TILES * 128), np.float32)
        nodes = prep["node_of"][c]              # [128, TILES] (-1 = pad)
        for t in range(TILES):
            ns = nodes[:LANES, t]
            x_fm[:, t * 128:t * 128 + LANES] = x[ns].T
        m = {
            "x_fm": x_fm,
            "idx_in": prep["idx_wrapped"][c],
            "dinv_in": prep["dinv_col"][c],
            "iden_in": np.eye(128, dtype=np.float32),
            "wl": np.asarray(inputs["Wl"], np.float32),
            "bl": np.asarray(inputs["bl"], np.float32).reshape(C_OUT, 1),
        }
        for k in range(4):
            m[f"w{k+1}"] = np.asarray(inputs[f"W{k+1}"], np.float32)
            m[f"b{k+1}"] = np.tile(
                np.asarray(inputs[f"b{k+1}"], np.float32)[None, :], (128, 1))
        maps.append(m)
    return maps


def _unshard(prep, results):
    out = np.empty((N, C_OUT), np.float32)
    for c in range(M):
        o = results[c]["out_fm"]                # [C_OUT, TILES*128]
        nodes = prep["node_of"][c]
        for t in range(TILES):
            ns = nodes[:LANES, t]
            out[ns] = o[:, t * 128:t * 128 + LANES].T
    return out


_CACHE = {}


def _get_program(edge_index, reps=1):
    key = (hash(edge_index.tobytes()), reps)
    if key not in _CACHE:
        prep = _prep(edge_index)
        nc = _build(prep, reps=reps)
        _CACHE[key] = (prep, nc)
    return _CACHE[key]


def kernel(**inputs):
    from concourse.bass_utils import run_bass_kernel_spmd

    edge_index = np.asarray(inputs["edge_index"], np.int32)
    reps = int(os.environ.get("GNN_REPS", "1"))
    prep, nc = _get_program(edge_index, reps)
    maps = _in_maps(prep, inputs)
    res = run_bass_kernel_spmd(nc, maps, core_ids=list(range(M)))
    kernel.last_results = res
    return _unshard(prep, res.results)

